# revision 18
# baseline (speedup 1.0000x reference)
"""AttentionCritic Trainium2 kernel — 8-core SPMD, head/query-half sharded,
bf16 compute with fp32 PSUM accumulation.

Math restructuring (exact up to fp assoc./bf16 rounding):
  mask[i,j] = (|x_i-x_j|<=4)&(|y_i-y_j|<=2)&(j>i)          (host, from int state)
  C' = [act(128), obs(16), 1]  (ones feature folds all biases)
  S_h = C' G' C'^T / 12,  G' = Aq Ak^T host-folded from the two-stage
        reference projections (C@Wq+bq)@Wiq+biq -> single eff mats + bias rows
  T1aug = C' [G' | Wv_aug]  computed X-form: T1aug^T = G''^T [h;a]^T + bias
        (G'' absorbs the obs encoder; bias row added in the PSUM->SBUF copy).
        Wv_aug = Av (Wo_eff Wduel)_head, so T1aug's tail also carries
        u^T = (C' Wv_aug)^T — the v/ctx/out-proj fold rides along for free.
  E_h = exp(S_h)  (softmax ratio is shift-invariant; |S/12|<~3 so bf16
        logits cost <~1% on exp)
  E^T via PE transpose;  u via PE transpose of T1aug tail rows
  D[j,i] = sum_k E[j,k] mask[i,k] + 1e-9 (eps preloaded in PSUM)
  R = mask^T * approx_recip(D);  W[k,i] = mask[i,k] * sum_j E[j,k] R[j,i]
  Q_p^T[a,i] = sum_k u[k,a] W[k,i]
  Q = sum_cores Q_p^T + n_i*c1 + c2  (host)

Sharding: core c handles (head h=c//2, query-half jm=c%2). Everything after
exp is linear in j and h, so each core emits a partial Q^T [5,256] over ALL
256 agents and the host sums the 8 partials. The j-half selection is uniform
across cores: the per-core input packing rotates the agent axis by 128*jm,
so slice [0:128] is always "my" j-half.

Per-core: 17 matmuls, ~390KB DMA, all matmuls bf16 (1 cyc/row — fp32r runs
4x slower as fp32_mode=HIGH on this part), accumulation in fp32 PSUM.
"""

import sys

for _p in ("/opt/trn_rl_repo",):
    if _p not in sys.path:
        sys.path.append(_p)

import contextlib

import numpy as np
import ml_dtypes

import concourse.bass as bass
import concourse.bacc as bacc
import concourse.mybir as mybir
from concourse.tile import TileContext
from concourse import bass_utils

N, HID, ACT, NH = 256, 128, 5, 4
D, E, HD = 144, 576, 144
NCORES = 8
F32 = mybir.dt.float32
BF16 = mybir.dt.bfloat16
BF16NP = ml_dtypes.bfloat16
SCALE = 1.0 / 12.0
CF = 145   # C' feature dim: act(128) + obs(16) + ones(1)
# T1aug^T tail tile layout: rows 0:8 = u^T (padded from 5), rows 8:32 zero
# pad (transpose/matmul partition starts must be 32-aligned), rows 32:49 =
# T1 tail features (obs 16 + ones 1)
TAUG = 177  # G'aug cols: main(128) + u(8) + pad(24) + T1tail(17)
TTL = 49    # tail tile partitions

# blob1 [128, B1_COLS] bf16 column layout (host packing must match)
# dma1 (sync):   GA(177) hTp(256) bT1m(1) btail(1)      -> 0:435
# dma2 (scalar): GB(177) aTp(256) wenc(16) benc(1)      -> 435:885
# dma3 (gpsimd): mT0(256) mT1(256) id128(128)           -> 885:1525
B1_GA, B1_HT, B1_BT1M, B1_BTL = 0, 177, 433, 434
B1_D1 = 435
B1_GB, B1_AT, B1_WENC, B1_BENC = 435, 612, 868, 884
B1_D2 = 885
B1_MT0, B1_MT1, B1_ID = 885, 1141, 1397
B1_COLS = 1525


def _build():
    nc = bacc.Bacc(target_bir_lowering=False)

    b1_d = nc.declare_dram_parameter("blob1", [128, B1_COLS], BF16, False)
    b2_d = nc.declare_dram_parameter("blob2", [128, 3], F32, False)
    out_d = nc.declare_dram_parameter("out", [ACT, N], BF16, True)

    with TileContext(nc) as tc:
        with contextlib.ExitStack() as ctx:
            wp = ctx.enter_context(tc.tile_pool(name="wp", bufs=1))
            pp = ctx.enter_context(tc.tile_pool(name="pp", bufs=8, space="PSUM"))

            def wt(shape, tag, dtype=BF16):
                return wp.tile(shape, dtype, tag=tag, name=tag)

            def ps(shape, dtype=F32):
                return pp.tile(shape, dtype, tag="mm", name="mm")

            b1 = wt([128, B1_COLS], "b1")
            nc.sync.dma_start(out=b1[:, 0:B1_D1], in_=b1_d[:, 0:B1_D1])
            nc.scalar.dma_start(out=b1[:, B1_D1:B1_D2],
                                in_=b1_d[:, B1_D1:B1_D2])
            b2 = wt([128, 3], "b2", F32)
            nc.gpsimd.dma_start(out=b2, in_=b2_d[:, :])
            nc.gpsimd.dma_start(out=b1[:, B1_D2:B1_COLS],
                                in_=b1_d[:, B1_D2:B1_COLS])

            GA = b1[:, B1_GA:B1_GA + TAUG]
            hTp = b1[:, B1_HT:B1_HT + N]
            bT1m = b2[:, 0:1]
            btail = b2[0:TTL, 1:2]
            GB = b1[:, B1_GB:B1_GB + TAUG]
            aTp = b1[:, B1_AT:B1_AT + N]
            wenc = b1[:, B1_WENC:B1_WENC + 16]
            benc = b2[0:16, 2:3]
            mT = [b1[:, B1_MT0:B1_MT0 + N], b1[:, B1_MT1:B1_MT1 + N]]
            ident = b1[:, B1_ID:B1_ID + 128]

            # ---------- T1aug^T = G''aug^T [h;a]^T + bias ----------
            # obs+ones block lives at partitions 32:49 so S pass-2's lhsT
            # (T1aug tail rows 32:49) and rhs share a base partition
            ctT_full = wt([64, N], "ctT")
            nc.vector.memset(ctT_full, 1.0)
            ctT = ctT_full[32:49, :]
            pT1m = ps([128, N])
            nc.tensor.matmul(pT1m, GA[:, 0:128], hTp, start=True, stop=False)
            nc.tensor.matmul(pT1m, GB[:, 0:128], aTp, start=False, stop=True)
            pT1t = ps([TTL, N])
            nc.tensor.matmul(pT1t, GA[:, 128:TAUG], hTp, start=True, stop=False)
            nc.tensor.matmul(pT1t, GB[:, 128:TAUG], aTp, start=False, stop=True)
            pObs = ps([16, N])
            nc.tensor.matmul(pObs, wenc, hTp, start=True, stop=True)
            T1m = wt([128, N], "T1m")
            nc.vector.tensor_scalar(T1m, pT1m, bT1m, None, mybir.AluOpType.add)
            T1t = wt([TTL, N], "T1t")
            nc.vector.tensor_scalar(T1t, pT1t, btail, None, mybir.AluOpType.add)
            nc.scalar.activation(ctT_full[32:48, :], pObs,
                                 mybir.ActivationFunctionType.Identity,
                                 bias=benc, scale=1.0)

            # ---------- S = T1_jhalf C'^T -> E = exp(S/12) (2 halves) ----
            pS = ps([128, N])
            nc.tensor.matmul(pS, T1m[:, 0:128], aTp, start=True, stop=False)
            nc.tensor.matmul(pS, T1t[32:TTL, 0:128], ctT, start=False, stop=True)
            Et = wt([128, N], "Et")
            nc.scalar.activation(Et[:, 0:128], pS[:, 0:128],
                                 mybir.ActivationFunctionType.Exp, scale=SCALE)
            nc.scalar.activation(Et[:, 128:N], pS[:, 128:N],
                                 mybir.ActivationFunctionType.Exp, scale=SCALE)

            # ---------- u via PE transpose of T1aug tail rows 0:8 ----------
            u_t = []
            for kc in range(2):
                pu = ps([128, 8], BF16)
                nc.tensor.transpose(pu, T1t[0:8, kc * 128:(kc + 1) * 128],
                                    ident[0:8, 0:8])
                t = wt([128, 8], f"u{kc}")
                nc.vector.tensor_copy(out=t, in_=pu)
                u_t.append(t)

            # ---------- E^T via PE transpose (exp(S)^T == exp(S^T)) ----------
            ET = []
            for kc in range(2):
                pT = ps([128, 128], BF16)
                nc.tensor.transpose(pT, Et[:, kc * 128:(kc + 1) * 128], ident)
                t = wt([128, 128], f"ET{kc}")
                nc.vector.tensor_copy(out=t, in_=pT)
                ET.append(t)

            # ---------- D; R = mask^T * approx_recip(D) ----
            # (no eps clamp: D==0 only for all-zero mask columns; the junk
            # recip produces there stays confined to those output columns,
            # which the host overwrites with the exact constant c2)
            pD = ps([128, N])
            nc.tensor.matmul(pD, ET[0], mT[0], start=True, stop=False)
            nc.tensor.matmul(pD, ET[1], mT[1], start=False, stop=True)
            Rr = wt([128, N], "Rr", F32)
            nc.vector.reciprocal_approx_fast(out=Rr, in_=pD)
            R = wt([128, N], "R")
            nc.vector.tensor_tensor(R, Rr, mT[0], mybir.AluOpType.mult)

            # ---------- W = mask^T * (E^T-partial over my j-half) ----------
            Wt = []
            for kc in range(2):
                pW = ps([128, N])
                nc.tensor.matmul(pW, Et[:, kc * 128:(kc + 1) * 128], R,
                                 start=True, stop=True)
                t = wt([128, N], f"W{kc}")
                nc.vector.tensor_tensor(t, pW, mT[kc], mybir.AluOpType.mult)
                Wt.append(t)

            # ---------- partial Q^T = u^T-contract with W : [8,256] ----------
            pQ = ps([8, N])
            nc.tensor.matmul(pQ, u_t[0], Wt[0], start=True, stop=False)
            nc.tensor.matmul(pQ, u_t[1], Wt[1], start=False, stop=True)
            Qsb = wt([8, N], "Qsb")
            nc.vector.tensor_copy(out=Qsb, in_=pQ)
            nc.sync.dma_start(out=out_d[:, :], in_=Qsb[0:ACT, :])

    nc.compile()
    return nc


_NC_CACHE = {}


def _make_in_maps(inputs):
    f32 = np.float32
    g = lambda k: np.asarray(inputs[k], dtype=np.float64)

    hidden = np.asarray(inputs["hidden_state_n"], dtype=f32)
    action = np.asarray(inputs["action_n"], dtype=f32)
    state = np.asarray(inputs["state_n"]).astype(np.int64)

    # host-side weight folding (float64, cast at the end)
    Wq_eff = g("Wq") @ g("Wiq")
    bq_eff = g("bq") @ g("Wiq") + g("biq")
    Wk_eff = g("Wk") @ g("Wik")
    bk_eff = g("bk") @ g("Wik") + g("bik")
    Wv_eff = g("Wv") @ g("Wiv")
    bv_eff = g("bv") @ g("Wiv") + g("biv")
    Wo_eff = g("Wo_proj") @ g("W_O")          # [576,144]
    bo_eff = g("bo_proj") @ g("W_O")          # [144]
    W_adv = g("W_adv")
    W_Q = (g("W_val") @ np.ones((1, ACT)) + W_adv
           - (W_adv @ np.ones((ACT, ACT))) / ACT)              # [144,5]
    b_Q = g("b_val")[0] + g("b_adv") - g("b_adv").mean()       # [5]
    W_out = Wo_eff @ W_Q                                       # [576,5]
    c1 = (bo_eff @ W_Q).astype(f32)                            # [5]
    c2 = b_Q.astype(f32)                                       # [5]

    # mask from int state (host): mask[i,j] = j observed by i
    dx = np.abs(state[:, None, 0] - state[None, :, 0])
    dy = np.abs(state[:, None, 1] - state[None, :, 1])
    upper = np.arange(N)[None, :] > np.arange(N)[:, None]
    mask = ((dx <= 4) & (dy <= 2) & upper).astype(f32)         # [N,N]
    n_i = mask.sum(axis=1)                                     # [N]
    maskT = np.ascontiguousarray(mask.T)                       # [j,i]

    W_enc = g("W_enc")                                         # [128,16]
    b_enc = np.asarray(inputs["b_enc"], dtype=f32)             # [16]
    hT = np.ascontiguousarray(hidden.T)                        # [128,256]
    aT = np.ascontiguousarray(action.T)
    bf = lambda a: np.ascontiguousarray(np.asarray(a, np.float32)
                                        .astype(BF16NP))

    in_maps = []
    for c in range(NCORES):
        h, jm = c // 2, c % 2
        perm = np.roll(np.arange(N), -jm * 128)
        cols = slice(144 * h, 144 * h + 144)

        # A-mats in C'-feature row order [act(128), obs(16), ones(1)]
        def amat(W, b):
            Wh, bh = W[:, cols], b[cols]
            return np.vstack([Wh[16:144], Wh[0:16], bh[None, :]])  # [145,144]
        Aq, Ak, Av = amat(Wq_eff, bq_eff), amat(Wk_eff, bk_eff), \
            amat(Wv_eff, bv_eff)
        Gp = Aq @ Ak.T                                         # [145,145]
        Wv_aug = np.concatenate([Av @ W_out[cols, :],
                                 np.zeros((CF, 3))], axis=1)   # [145,8]
        # G'aug cols: T1 main(128) | u(8) | pad(24) | T1 tail(17)
        Gaug = np.concatenate([Gp[:, 0:128], Wv_aug,
                               np.zeros((CF, 24)), Gp[:, 128:145]], axis=1)
        GppA = W_enc @ Gaug[128:144, :]                        # hid rows [128,177]
        GppB = Gaug[0:128, :]                                  # act rows [128,177]
        # ones-row of G'aug PLUS the obs-encoder bias pushed through the
        # obs rows (b_enc contributes obs = h@W_enc + b_enc to T1's X-form)
        bT1a = Gaug[144, :] + g("b_enc") @ Gaug[128:144, :]    # [177]
        btail = np.zeros((128, 1))
        btail[0:TTL, 0] = bT1a[128:TAUG]
        mTp = maskT[perm, :]
        benc_col = np.zeros((128, 1))
        benc_col[0:16, 0] = b_enc

        b1 = np.concatenate([
            bf(GppA), bf(hT[:, perm]),
            bf(bT1a[0:128].reshape(128, 1)), bf(btail),
            bf(GppB), bf(aT[:, perm]), bf(W_enc), bf(benc_col),
            bf(mTp[0:128]), bf(mTp[128:256]),
            bf(np.eye(128))], axis=1)
        b2c = np.concatenate([bT1a[0:128].reshape(128, 1), btail,
                              benc_col], axis=1).astype(f32)
        in_maps.append({"blob1": np.ascontiguousarray(b1, dtype=BF16NP),
                        "blob2": np.ascontiguousarray(b2c, dtype=f32)})
    return in_maps, n_i, c1, c2


def kernel(**inputs):
    if "nc" not in _NC_CACHE:
        _NC_CACHE["nc"] = _build()
    nc = _NC_CACHE["nc"]
    in_maps, n_i, c1, c2 = _make_in_maps(inputs)
    res = bass_utils.run_bass_kernel_spmd(nc, in_maps, core_ids=list(range(NCORES)))
    QT = np.zeros((ACT, N), np.float32)
    for c in range(NCORES):
        QT += np.asarray(res.results[c]["out"], np.float32)
    Q = QT.T + n_i[:, None] * c1[None, :] + c2[None, :]
    Q[n_i == 0] = c2  # agents observing nobody: exact constant (junk-proof)
    return Q.astype(np.float32)


# revision 20
# speedup vs baseline: 1.0250x; 1.0250x over previous
"""AttentionCritic Trainium2 kernel — 8-core SPMD, head/query-half sharded,
bf16 compute with fp32 PSUM accumulation.

Math restructuring (exact up to fp assoc./bf16 rounding):
  mask[i,j] = (|x_i-x_j|<=4)&(|y_i-y_j|<=2)&(j>i)          (host, from int state)
  C' = [act(128), obs(16), 1]  (ones feature folds all biases)
  S_h = C' G' C'^T / 12,  G' = Aq Ak^T host-folded from the two-stage
        reference projections (C@Wq+bq)@Wiq+biq -> single eff mats + bias rows
  T1aug = C' [G' | Wv_aug]  computed X-form: T1aug^T = G''^T [h;a]^T + bias
        (G'' absorbs the obs encoder; bias row added in the PSUM->SBUF copy).
        Wv_aug = Av (Wo_eff Wduel)_head, so T1aug's tail also carries
        u^T = (C' Wv_aug)^T — the v/ctx/out-proj fold rides along for free.
  E_h = exp(S_h)  (softmax ratio is shift-invariant; |S/12|<~3 so bf16
        logits cost <~1% on exp)
  E^T via PE transpose;  u via PE transpose of T1aug tail rows
  D[j,i] = sum_k E[j,k] mask[i,k] + 1e-9 (eps preloaded in PSUM)
  R = mask^T * approx_recip(D);  W[k,i] = mask[i,k] * sum_j E[j,k] R[j,i]
  Q_p^T[a,i] = sum_k u[k,a] W[k,i]
  Q = sum_cores Q_p^T + n_i*c1 + c2  (host)

Sharding: core c handles (head h=c//2, query-half jm=c%2). Everything after
exp is linear in j and h, so each core emits a partial Q^T [5,256] over ALL
256 agents and the host sums the 8 partials. The j-half selection is uniform
across cores: the per-core input packing rotates the agent axis by 128*jm,
so slice [0:128] is always "my" j-half.

Per-core: 17 matmuls, ~390KB DMA, all matmuls bf16 (1 cyc/row — fp32r runs
4x slower as fp32_mode=HIGH on this part), accumulation in fp32 PSUM.
"""

import sys

for _p in ("/opt/trn_rl_repo",):
    if _p not in sys.path:
        sys.path.append(_p)

import contextlib

import numpy as np
import ml_dtypes

import concourse.bass as bass
import concourse.bacc as bacc
import concourse.mybir as mybir
from concourse.tile import TileContext
from concourse import bass_utils

N, HID, ACT, NH = 256, 128, 5, 4
D, E, HD = 144, 576, 144
NCORES = 8
F32 = mybir.dt.float32
BF16 = mybir.dt.bfloat16
BF16NP = ml_dtypes.bfloat16
SCALE = 1.0 / 12.0
CF = 145   # C' feature dim: act(128) + obs(16) + ones(1)
# T1aug^T tail tile layout: rows 0:8 = u^T (padded from 5), rows 8:32 zero
# pad (transpose/matmul partition starts must be 32-aligned), rows 32:49 =
# T1 tail features (obs 16 + ones 1)
TAUG = 177  # G'aug cols: main(128) + u(8) + pad(24) + T1tail(17)
TTL = 49    # tail tile partitions

# blob1 [128, B1_COLS] bf16 column layout (host packing must match)
# dma1 (sync):   GA(177) hTp(256) wenc(16)             -> 0:449
# dma2 (scalar): GB(177) aTp(256)                      -> 449:882
# dma3 (gpsimd): mT0(256) mT1(256) id128(128)          -> 882:1522
B1_GA, B1_HT, B1_WENC = 0, 177, 433
B1_D1 = 449
B1_GB, B1_AT = 449, 626
B1_D2 = 882
B1_MT0, B1_MT1, B1_ID = 882, 1138, 1394
B1_COLS = 1522


def _build():
    nc = bacc.Bacc(target_bir_lowering=False)

    b1_d = nc.declare_dram_parameter("blob1", [128, B1_COLS], BF16, False)
    b2_d = nc.declare_dram_parameter("blob2", [128, 3], F32, False)
    out_d = nc.declare_dram_parameter("out", [ACT, N], BF16, True)

    with TileContext(nc) as tc:
        with contextlib.ExitStack() as ctx:
            wp = ctx.enter_context(tc.tile_pool(name="wp", bufs=1))
            pp = ctx.enter_context(tc.tile_pool(name="pp", bufs=8, space="PSUM"))

            def wt(shape, tag, dtype=BF16):
                return wp.tile(shape, dtype, tag=tag, name=tag)

            def ps(shape, dtype=F32):
                return pp.tile(shape, dtype, tag="mm", name="mm")

            b1 = wt([128, B1_COLS], "b1")
            nc.sync.dma_start(out=b1[:, 0:B1_D1], in_=b1_d[:, 0:B1_D1])
            nc.scalar.dma_start(out=b1[:, B1_D1:B1_D2],
                                in_=b1_d[:, B1_D1:B1_D2])
            b2 = wt([128, 3], "b2", F32)
            nc.gpsimd.dma_start(out=b2, in_=b2_d[:, :])
            nc.gpsimd.dma_start(out=b1[:, B1_D2:B1_COLS],
                                in_=b1_d[:, B1_D2:B1_COLS])

            GA = b1[:, B1_GA:B1_GA + TAUG]
            hTp = b1[:, B1_HT:B1_HT + N]
            bT1m = b2[:, 0:1]
            btail = b2[0:TTL, 1:2]
            GB = b1[:, B1_GB:B1_GB + TAUG]
            aTp = b1[:, B1_AT:B1_AT + N]
            wenc = b1[:, B1_WENC:B1_WENC + 16]
            benc = b2[0:16, 2:3]

            mT = [b1[:, B1_MT0:B1_MT0 + N], b1[:, B1_MT1:B1_MT1 + N]]
            ident = b1[:, B1_ID:B1_ID + 128]

            # ---------- T1aug^T = G''aug^T [h;a]^T + bias ----------
            # obs+ones block lives at partitions 32:49 so S pass-2's lhsT
            # (T1aug tail rows 32:49) and rhs share a base partition
            ctT_full = wt([64, N], "ctT")
            nc.vector.memset(ctT_full, 1.0)
            ctT = ctT_full[32:49, :]
            pObs = ps([16, N])
            nc.tensor.matmul(pObs, wenc, hTp, start=True, stop=True)
            pT1m = ps([128, N])
            nc.tensor.matmul(pT1m, GA[:, 0:128], hTp, start=True, stop=False)
            nc.tensor.matmul(pT1m, GB[:, 0:128], aTp, start=False, stop=True)
            pT1t = ps([TTL, N])
            nc.tensor.matmul(pT1t, GA[:, 128:TAUG], hTp, start=True, stop=False)
            nc.tensor.matmul(pT1t, GB[:, 128:TAUG], aTp, start=False, stop=True)
            T1m = wt([128, N], "T1m")
            nc.vector.tensor_scalar(T1m, pT1m, bT1m, None, mybir.AluOpType.add)
            T1t = wt([TTL, N], "T1t")
            nc.vector.tensor_scalar(T1t, pT1t, btail, None, mybir.AluOpType.add)
            nc.scalar.activation(ctT_full[32:48, :], pObs,
                                 mybir.ActivationFunctionType.Identity,
                                 bias=benc, scale=1.0)

            # ---------- S = T1_jhalf C'^T -> E = exp(S/12) (2 halves) ----
            pS = ps([128, N])
            nc.tensor.matmul(pS, T1m[:, 0:128], aTp, start=True, stop=False)
            nc.tensor.matmul(pS, T1t[32:TTL, 0:128], ctT, start=False, stop=True)
            Et = wt([128, N], "Et")
            nc.scalar.activation(Et[:, 0:128], pS[:, 0:128],
                                 mybir.ActivationFunctionType.Exp, scale=SCALE)
            nc.scalar.activation(Et[:, 128:N], pS[:, 128:N],
                                 mybir.ActivationFunctionType.Exp, scale=SCALE)

            # ---------- u via PE transpose of T1aug tail rows 0:8 ----------
            u_t = []
            for kc in range(2):
                pu = ps([128, 8], BF16)
                nc.tensor.transpose(pu, T1t[0:8, kc * 128:(kc + 1) * 128],
                                    ident[0:8, 0:8])
                t = wt([128, 8], f"u{kc}")
                nc.vector.tensor_copy(out=t, in_=pu)
                u_t.append(t)

            # ---------- E^T via PE transpose (exp(S)^T == exp(S^T)) ----------
            pTT = ps([128, N], BF16)
            nc.tensor.transpose(pTT[:, 0:128], Et[:, 0:128], ident)
            nc.tensor.transpose(pTT[:, 128:N], Et[:, 128:N], ident)
            ETb = wt([128, N], "ETb")
            nc.vector.tensor_copy(out=ETb, in_=pTT)
            ET = [ETb[:, 0:128], ETb[:, 128:N]]

            # ---------- D; R = mask^T * approx_recip(D) ----
            # (no eps clamp: D==0 only for all-zero mask columns; the junk
            # recip produces there stays confined to those output columns,
            # which the host overwrites with the exact constant c2)
            pD = ps([128, N])
            nc.tensor.matmul(pD, ET[0], mT[0], start=True, stop=False)
            nc.tensor.matmul(pD, ET[1], mT[1], start=False, stop=True)
            Rr = wt([128, N], "Rr", F32)
            nc.vector.reciprocal_approx_fast(out=Rr, in_=pD)
            R = wt([128, N], "R")
            nc.vector.tensor_tensor(R, Rr, mT[0], mybir.AluOpType.mult)

            # ---------- W = mask^T * (E^T-partial over my j-half) ----------
            Wt = []
            for kc in range(2):
                pW = ps([128, N])
                nc.tensor.matmul(pW, Et[:, kc * 128:(kc + 1) * 128], R,
                                 start=True, stop=True)
                t = wt([128, N], f"W{kc}")
                nc.vector.tensor_tensor(t, pW, mT[kc], mybir.AluOpType.mult)
                Wt.append(t)

            # ---------- partial Q^T = u^T-contract with W : [8,256] ----------
            pQ = ps([8, N])
            nc.tensor.matmul(pQ, u_t[0], Wt[0], start=True, stop=False)
            nc.tensor.matmul(pQ, u_t[1], Wt[1], start=False, stop=True)
            Qsb = wt([8, N], "Qsb")
            nc.vector.tensor_copy(out=Qsb, in_=pQ)
            nc.sync.dma_start(out=out_d[:, :], in_=Qsb[0:ACT, :])

    nc.compile()
    return nc


_NC_CACHE = {}


def _make_in_maps(inputs):
    f32 = np.float32
    g = lambda k: np.asarray(inputs[k], dtype=np.float64)

    hidden = np.asarray(inputs["hidden_state_n"], dtype=f32)
    action = np.asarray(inputs["action_n"], dtype=f32)
    state = np.asarray(inputs["state_n"]).astype(np.int64)

    # host-side weight folding (float64, cast at the end)
    Wq_eff = g("Wq") @ g("Wiq")
    bq_eff = g("bq") @ g("Wiq") + g("biq")
    Wk_eff = g("Wk") @ g("Wik")
    bk_eff = g("bk") @ g("Wik") + g("bik")
    Wv_eff = g("Wv") @ g("Wiv")
    bv_eff = g("bv") @ g("Wiv") + g("biv")
    Wo_eff = g("Wo_proj") @ g("W_O")          # [576,144]
    bo_eff = g("bo_proj") @ g("W_O")          # [144]
    W_adv = g("W_adv")
    W_Q = (g("W_val") @ np.ones((1, ACT)) + W_adv
           - (W_adv @ np.ones((ACT, ACT))) / ACT)              # [144,5]
    b_Q = g("b_val")[0] + g("b_adv") - g("b_adv").mean()       # [5]
    W_out = Wo_eff @ W_Q                                       # [576,5]
    c1 = (bo_eff @ W_Q).astype(f32)                            # [5]
    c2 = b_Q.astype(f32)                                       # [5]

    # mask from int state (host): mask[i,j] = j observed by i
    dx = np.abs(state[:, None, 0] - state[None, :, 0])
    dy = np.abs(state[:, None, 1] - state[None, :, 1])
    upper = np.arange(N)[None, :] > np.arange(N)[:, None]
    mask = ((dx <= 4) & (dy <= 2) & upper).astype(f32)         # [N,N]
    n_i = mask.sum(axis=1)                                     # [N]
    maskT = np.ascontiguousarray(mask.T)                       # [j,i]

    W_enc = g("W_enc")                                         # [128,16]
    b_enc = np.asarray(inputs["b_enc"], dtype=f32)             # [16]
    hT = np.ascontiguousarray(hidden.T)                        # [128,256]
    aT = np.ascontiguousarray(action.T)
    bf = lambda a: np.ascontiguousarray(np.asarray(a, np.float32)
                                        .astype(BF16NP))

    in_maps = []
    for c in range(NCORES):
        h, jm = c // 2, c % 2
        perm = np.roll(np.arange(N), -jm * 128)
        cols = slice(144 * h, 144 * h + 144)

        # A-mats in C'-feature row order [act(128), obs(16), ones(1)]
        def amat(W, b):
            Wh, bh = W[:, cols], b[cols]
            return np.vstack([Wh[16:144], Wh[0:16], bh[None, :]])  # [145,144]
        Aq, Ak, Av = amat(Wq_eff, bq_eff), amat(Wk_eff, bk_eff), \
            amat(Wv_eff, bv_eff)
        Gp = Aq @ Ak.T                                         # [145,145]
        Wv_aug = np.concatenate([Av @ W_out[cols, :],
                                 np.zeros((CF, 3))], axis=1)   # [145,8]
        # G'aug cols: T1 main(128) | u(8) | pad(24) | T1 tail(17)
        Gaug = np.concatenate([Gp[:, 0:128], Wv_aug,
                               np.zeros((CF, 24)), Gp[:, 128:145]], axis=1)
        GppA = W_enc @ Gaug[128:144, :]                        # hid rows [128,177]
        GppB = Gaug[0:128, :]                                  # act rows [128,177]
        # ones-row of G'aug PLUS the obs-encoder bias pushed through the
        # obs rows (b_enc contributes obs = h@W_enc + b_enc to T1's X-form)
        bT1a = Gaug[144, :] + g("b_enc") @ Gaug[128:144, :]    # [177]
        btail = np.zeros((128, 1))
        btail[0:TTL, 0] = bT1a[128:TAUG]
        mTp = maskT[perm, :]
        benc_col = np.zeros((128, 1))
        benc_col[0:16, 0] = b_enc

        b1 = np.concatenate([
            bf(GppA), bf(hT[:, perm]), bf(W_enc),
            bf(GppB), bf(aT[:, perm]),
            bf(mTp[0:128]), bf(mTp[128:256]),
            bf(np.eye(128))], axis=1)
        b2c = np.concatenate([bT1a[0:128].reshape(128, 1), btail,
                              benc_col], axis=1).astype(f32)
        in_maps.append({"blob1": np.ascontiguousarray(b1, dtype=BF16NP),
                        "blob2": np.ascontiguousarray(b2c, dtype=f32)})
    return in_maps, n_i, c1, c2


def kernel(**inputs):
    if "nc" not in _NC_CACHE:
        _NC_CACHE["nc"] = _build()
    nc = _NC_CACHE["nc"]
    in_maps, n_i, c1, c2 = _make_in_maps(inputs)
    res = bass_utils.run_bass_kernel_spmd(nc, in_maps, core_ids=list(range(NCORES)))
    QT = np.zeros((ACT, N), np.float32)
    for c in range(NCORES):
        QT += np.asarray(res.results[c]["out"], np.float32)
    Q = QT.T + n_i[:, None] * c1[None, :] + c2[None, :]
    Q[n_i == 0] = c2  # agents observing nobody: exact constant (junk-proof)
    return Q.astype(np.float32)


# revision 21
# speedup vs baseline: 1.0613x; 1.0355x over previous
"""AttentionCritic Trainium2 kernel — 8-core SPMD, head/query-half sharded,
bf16 compute with fp32 PSUM accumulation.

Math restructuring (exact up to fp assoc./bf16 rounding):
  mask[i,j] = (|x_i-x_j|<=4)&(|y_i-y_j|<=2)&(j>i)          (host, from int state)
  C' = [act(128), obs(16), 1]  (ones feature folds all biases)
  S_h = C' G' C'^T / 12,  G' = Aq Ak^T host-folded from the two-stage
        reference projections (C@Wq+bq)@Wiq+biq -> single eff mats + bias rows
  T1aug = C' [G' | Wv_aug]  computed X-form: T1aug^T = G''^T [h;a]^T + bias
        (G'' absorbs the obs encoder; bias row added in the PSUM->SBUF copy).
        Wv_aug = Av (Wo_eff Wduel)_head, so T1aug's tail also carries
        u^T = (C' Wv_aug)^T — the v/ctx/out-proj fold rides along for free.
  E_h = exp(S_h)  (softmax ratio is shift-invariant; |S/12|<~3 so bf16
        logits cost <~1% on exp)
  E^T via PE transpose;  u via PE transpose of T1aug tail rows
  D[j,i] = sum_k E[j,k] mask[i,k] + 1e-9 (eps preloaded in PSUM)
  R = mask^T * approx_recip(D);  W[k,i] = mask[i,k] * sum_j E[j,k] R[j,i]
  Q_p^T[a,i] = sum_k u[k,a] W[k,i]
  Q = sum_cores Q_p^T + n_i*c1 + c2  (host)

Sharding: core c handles (head h=c//2, query-half jm=c%2). Everything after
exp is linear in j and h, so each core emits a partial Q^T [5,256] over ALL
256 agents and the host sums the 8 partials. The j-half selection is uniform
across cores: the per-core input packing rotates the agent axis by 128*jm,
so slice [0:128] is always "my" j-half.

Per-core: 17 matmuls, ~390KB DMA, all matmuls bf16 (1 cyc/row — fp32r runs
4x slower as fp32_mode=HIGH on this part), accumulation in fp32 PSUM.
"""

import sys

for _p in ("/opt/trn_rl_repo",):
    if _p not in sys.path:
        sys.path.append(_p)

import contextlib

import numpy as np
import ml_dtypes

import concourse.bass as bass
import concourse.bacc as bacc
import concourse.mybir as mybir
from concourse.tile import TileContext
from concourse import bass_utils

N, HID, ACT, NH = 256, 128, 5, 4
D, E, HD = 144, 576, 144
NCORES = 8
F32 = mybir.dt.float32
BF16 = mybir.dt.bfloat16
BF16NP = ml_dtypes.bfloat16
SCALE = 1.0 / 12.0
CF = 145   # C' feature dim: act(128) + obs(16) + ones(1)
# T1aug^T tail tile layout: rows 0:8 = u^T (padded from 5), rows 8:32 zero
# pad (transpose/matmul partition starts must be 32-aligned), rows 32:49 =
# T1 tail features (obs 16 + ones 1)
TAUG = 177  # G'aug cols: main(128) + u(8) + pad(24) + T1tail(17)
TTL = 49    # tail tile partitions

# blob1 [128, B1_COLS] bf16 column layout (host packing must match)
# dma1 (sync):   GA(177) hTp(256) wenc(16)             -> 0:449
# dma2 (scalar): GB(177) aTp(256)                      -> 449:882
# dma3 (gpsimd): mT0(256) mT1(256) id128(128)          -> 882:1522
B1_GA, B1_HT, B1_WENC = 0, 177, 433
B1_D1 = 449
B1_GB, B1_AT = 449, 626
B1_D2 = 882
B1_MT0, B1_MT1, B1_ID = 882, 1138, 1394
B1_COLS = 1522


def _build():
    nc = bacc.Bacc(target_bir_lowering=False)

    b1_d = nc.declare_dram_parameter("blob1", [128, B1_COLS], BF16, False)
    b2_d = nc.declare_dram_parameter("blob2", [128, 3], F32, False)
    out_d = nc.declare_dram_parameter("out", [ACT, N], BF16, True)

    with TileContext(nc) as tc:
        with contextlib.ExitStack() as ctx:
            wp = ctx.enter_context(tc.tile_pool(name="wp", bufs=1))
            pp = ctx.enter_context(tc.tile_pool(name="pp", bufs=8, space="PSUM"))

            def wt(shape, tag, dtype=BF16):
                return wp.tile(shape, dtype, tag=tag, name=tag)

            def ps(shape, dtype=F32):
                return pp.tile(shape, dtype, tag="mm", name="mm")

            b1 = wt([128, B1_COLS], "b1")
            nc.sync.dma_start(out=b1[:, 0:B1_D1], in_=b1_d[:, 0:B1_D1])
            nc.scalar.dma_start(out=b1[:, B1_D1:B1_D2],
                                in_=b1_d[:, B1_D1:B1_D2])
            b2 = wt([128, 3], "b2", F32)
            nc.gpsimd.dma_start(out=b2, in_=b2_d[:, :])
            nc.gpsimd.dma_start(out=b1[:, B1_D2:B1_COLS],
                                in_=b1_d[:, B1_D2:B1_COLS])

            GA = b1[:, B1_GA:B1_GA + TAUG]
            hTp = b1[:, B1_HT:B1_HT + N]
            bT1m = b2[:, 0:1]
            btail = b2[0:TTL, 1:2]
            GB = b1[:, B1_GB:B1_GB + TAUG]
            aTp = b1[:, B1_AT:B1_AT + N]
            wenc = b1[:, B1_WENC:B1_WENC + 16]
            benc = b2[0:16, 2:3]

            mT = [b1[:, B1_MT0:B1_MT0 + N], b1[:, B1_MT1:B1_MT1 + N]]
            ident = b1[:, B1_ID:B1_ID + 128]

            # ---------- T1aug^T = G''aug^T [h;a]^T + bias ----------
            # obs+ones block lives at partitions 32:49 so S pass-2's lhsT
            # (T1aug tail rows 32:49) and rhs share a base partition
            ctT_full = wt([64, N], "ctT")
            nc.vector.memset(ctT_full, 1.0)
            ctT = ctT_full[32:49, :]
            pObs = ps([16, N])
            nc.tensor.matmul(pObs, wenc, hTp, start=True, stop=True)
            pT1m = ps([128, N])
            nc.tensor.matmul(pT1m, GA[:, 0:128], hTp, start=True, stop=False)
            nc.tensor.matmul(pT1m, GB[:, 0:128], aTp, start=False, stop=True)
            pT1t = ps([TTL, N])
            nc.tensor.matmul(pT1t, GA[:, 128:TAUG], hTp, start=True, stop=False)
            nc.tensor.matmul(pT1t, GB[:, 128:TAUG], aTp, start=False, stop=True)
            T1m = wt([128, N], "T1m")
            nc.vector.tensor_scalar(T1m, pT1m, bT1m, None, mybir.AluOpType.add)
            T1t = wt([TTL, N], "T1t")
            nc.vector.tensor_scalar(T1t, pT1t, btail, None, mybir.AluOpType.add)
            nc.scalar.activation(ctT_full[32:48, :], pObs,
                                 mybir.ActivationFunctionType.Identity,
                                 bias=benc, scale=1.0)

            # ---------- S = T1_jhalf C'^T -> E = exp(S/12) (2 halves) ----
            pS = ps([128, N])
            nc.tensor.matmul(pS, T1m[:, 0:128], aTp, start=True, stop=False)
            nc.tensor.matmul(pS, T1t[32:TTL, 0:128], ctT, start=False, stop=True)
            Et = wt([128, N], "Et")
            nc.scalar.activation(Et[:, 0:128], pS[:, 0:128],
                                 mybir.ActivationFunctionType.Exp, scale=SCALE)
            nc.scalar.activation(Et[:, 128:N], pS[:, 128:N],
                                 mybir.ActivationFunctionType.Exp, scale=SCALE)

            # ---------- u via PE transpose of T1aug tail rows 0:8 ----------
            u_t = []
            for kc in range(2):
                pu = ps([128, 8], BF16)
                nc.tensor.transpose(pu, T1t[0:8, kc * 128:(kc + 1) * 128],
                                    ident[0:8, 0:8])
                t = wt([128, 8], f"u{kc}")
                nc.vector.tensor_copy(out=t, in_=pu)
                u_t.append(t)

            # ---------- E^T via PE transpose (exp(S)^T == exp(S^T)),
            # pipelined per k-half straight into D's accumulation group ----
            # (no eps clamp: D==0 only for all-zero mask columns; the junk
            # recip produces there stays confined to those output columns,
            # which the host overwrites with the exact constant c2)
            pT0 = ps([128, 128], BF16)
            nc.tensor.transpose(pT0, Et[:, 0:128], ident)
            pT1 = ps([128, 128], BF16)
            nc.tensor.transpose(pT1, Et[:, 128:N], ident)
            ET0 = wt([128, 128], "ET0")
            nc.vector.tensor_copy(out=ET0, in_=pT0)
            ET1 = wt([128, 128], "ET1")
            nc.vector.tensor_copy(out=ET1, in_=pT1)
            pD = ps([128, N])
            nc.tensor.matmul(pD, ET0, mT[0], start=True, stop=False)
            nc.tensor.matmul(pD, ET1, mT[1], start=False, stop=True)
            Rr = wt([128, N], "Rr", F32)
            nc.vector.reciprocal_approx_fast(out=Rr, in_=pD)
            R = wt([128, N], "R")
            nc.vector.tensor_tensor(R, Rr, mT[0], mybir.AluOpType.mult)

            # ---------- W = mask^T * (E^T-partial over my j-half) ----------
            Wt = []
            for kc in range(2):
                pW = ps([128, N])
                nc.tensor.matmul(pW, Et[:, kc * 128:(kc + 1) * 128], R,
                                 start=True, stop=True)
                t = wt([128, N], f"W{kc}")
                nc.vector.tensor_tensor(t, pW, mT[kc], mybir.AluOpType.mult)
                Wt.append(t)

            # ---------- partial Q^T = u^T-contract with W : [8,256] ----------
            pQ = ps([8, N])
            nc.tensor.matmul(pQ, u_t[0], Wt[0], start=True, stop=False)
            nc.tensor.matmul(pQ, u_t[1], Wt[1], start=False, stop=True)
            Qsb = wt([8, N], "Qsb")
            nc.vector.tensor_copy(out=Qsb, in_=pQ)
            nc.sync.dma_start(out=out_d[:, :], in_=Qsb[0:ACT, :])

    nc.compile()
    return nc


_NC_CACHE = {}


def _make_in_maps(inputs):
    f32 = np.float32
    g = lambda k: np.asarray(inputs[k], dtype=np.float64)

    hidden = np.asarray(inputs["hidden_state_n"], dtype=f32)
    action = np.asarray(inputs["action_n"], dtype=f32)
    state = np.asarray(inputs["state_n"]).astype(np.int64)

    # host-side weight folding (float64, cast at the end)
    Wq_eff = g("Wq") @ g("Wiq")
    bq_eff = g("bq") @ g("Wiq") + g("biq")
    Wk_eff = g("Wk") @ g("Wik")
    bk_eff = g("bk") @ g("Wik") + g("bik")
    Wv_eff = g("Wv") @ g("Wiv")
    bv_eff = g("bv") @ g("Wiv") + g("biv")
    Wo_eff = g("Wo_proj") @ g("W_O")          # [576,144]
    bo_eff = g("bo_proj") @ g("W_O")          # [144]
    W_adv = g("W_adv")
    W_Q = (g("W_val") @ np.ones((1, ACT)) + W_adv
           - (W_adv @ np.ones((ACT, ACT))) / ACT)              # [144,5]
    b_Q = g("b_val")[0] + g("b_adv") - g("b_adv").mean()       # [5]
    W_out = Wo_eff @ W_Q                                       # [576,5]
    c1 = (bo_eff @ W_Q).astype(f32)                            # [5]
    c2 = b_Q.astype(f32)                                       # [5]

    # mask from int state (host): mask[i,j] = j observed by i
    dx = np.abs(state[:, None, 0] - state[None, :, 0])
    dy = np.abs(state[:, None, 1] - state[None, :, 1])
    upper = np.arange(N)[None, :] > np.arange(N)[:, None]
    mask = ((dx <= 4) & (dy <= 2) & upper).astype(f32)         # [N,N]
    n_i = mask.sum(axis=1)                                     # [N]
    maskT = np.ascontiguousarray(mask.T)                       # [j,i]

    W_enc = g("W_enc")                                         # [128,16]
    b_enc = np.asarray(inputs["b_enc"], dtype=f32)             # [16]
    hT = np.ascontiguousarray(hidden.T)                        # [128,256]
    aT = np.ascontiguousarray(action.T)
    bf = lambda a: np.ascontiguousarray(np.asarray(a, np.float32)
                                        .astype(BF16NP))

    in_maps = []
    for c in range(NCORES):
        h, jm = c // 2, c % 2
        perm = np.roll(np.arange(N), -jm * 128)
        cols = slice(144 * h, 144 * h + 144)

        # A-mats in C'-feature row order [act(128), obs(16), ones(1)]
        def amat(W, b):
            Wh, bh = W[:, cols], b[cols]
            return np.vstack([Wh[16:144], Wh[0:16], bh[None, :]])  # [145,144]
        Aq, Ak, Av = amat(Wq_eff, bq_eff), amat(Wk_eff, bk_eff), \
            amat(Wv_eff, bv_eff)
        Gp = Aq @ Ak.T                                         # [145,145]
        Wv_aug = np.concatenate([Av @ W_out[cols, :],
                                 np.zeros((CF, 3))], axis=1)   # [145,8]
        # G'aug cols: T1 main(128) | u(8) | pad(24) | T1 tail(17)
        Gaug = np.concatenate([Gp[:, 0:128], Wv_aug,
                               np.zeros((CF, 24)), Gp[:, 128:145]], axis=1)
        GppA = W_enc @ Gaug[128:144, :]                        # hid rows [128,177]
        GppB = Gaug[0:128, :]                                  # act rows [128,177]
        # ones-row of G'aug PLUS the obs-encoder bias pushed through the
        # obs rows (b_enc contributes obs = h@W_enc + b_enc to T1's X-form)
        bT1a = Gaug[144, :] + g("b_enc") @ Gaug[128:144, :]    # [177]
        btail = np.zeros((128, 1))
        btail[0:TTL, 0] = bT1a[128:TAUG]
        mTp = maskT[perm, :]
        benc_col = np.zeros((128, 1))
        benc_col[0:16, 0] = b_enc

        b1 = np.concatenate([
            bf(GppA), bf(hT[:, perm]), bf(W_enc),
            bf(GppB), bf(aT[:, perm]),
            bf(mTp[0:128]), bf(mTp[128:256]),
            bf(np.eye(128))], axis=1)
        b2c = np.concatenate([bT1a[0:128].reshape(128, 1), btail,
                              benc_col], axis=1).astype(f32)
        in_maps.append({"blob1": np.ascontiguousarray(b1, dtype=BF16NP),
                        "blob2": np.ascontiguousarray(b2c, dtype=f32)})
    return in_maps, n_i, c1, c2


def kernel(**inputs):
    if "nc" not in _NC_CACHE:
        _NC_CACHE["nc"] = _build()
    nc = _NC_CACHE["nc"]
    in_maps, n_i, c1, c2 = _make_in_maps(inputs)
    res = bass_utils.run_bass_kernel_spmd(nc, in_maps, core_ids=list(range(NCORES)))
    QT = np.zeros((ACT, N), np.float32)
    for c in range(NCORES):
        QT += np.asarray(res.results[c]["out"], np.float32)
    Q = QT.T + n_i[:, None] * c1[None, :] + c2[None, :]
    Q[n_i == 0] = c2  # agents observing nobody: exact constant (junk-proof)
    return Q.astype(np.float32)
